# revision 24
# baseline (speedup 1.0000x reference)
"""Trainium2 Bass kernel for nn_MMHA_78039555768536.

Gated mix of per-segment causal softmax attention and a linear-attention
memory (delta rule, memory summed over batch per segment).

Strategy (8 cores): reformulate the memory recurrence as a linear matrix
recurrence  M_{t+1} = A_t M_t + B_t  with
    A_t = -(s*sk)^T (s*sk)  (A-part; s = rsqrt(d), scaled in place)
    B_t = sum_b sk_b^T v_b
    d_b = sk_b @ z_{b,t};  z is a prefix of column-sums of sk (M-independent)
Core c owns segments {2c, 2c+1} for all batches.  Two all-gathers:
 AG1: per-segment colsums of sk (for the z prefix)  [tiny]
 AG2: per-core pair composition (Abar^T, Bbar)      [1 MB bf16 per rank]
Then every core redundantly runs the 7-step pair chain and selects its own
prefix M via a per-core one-hot input (SPMD, no branches).

Perf structure (phase order chosen so the collectives hide behind
z-independent compute):
  A:   k+v projections, sk, colsums          -> AG1 triggers
  B_t: both segments (z-free)   + unit-0 q/kT/attention   [covers AG1]
  z prefix, d, in-place rsqrt(d) scale, A_t, pair compose -> AG2 triggers
  q/kT projections + attention for units 1..7             [covers AG2]
  phase-B prep (attention loads, memread denominators), 7-step chain,
  memread + combine + Wd.
Softmax/memread reciprocals are batched through DRAM into [128,32] tiles
(DVE reciprocal cost is free-size-bound: ~3.4us for [1,512] vs ~0.25us).
Attention is software-pipelined one head deep: scores/exp/mask of head h
overlap the attention-value matmuls of head h-1, keeping PE dense.
"""

import os
import sys

sys.path.insert(0, "/opt/trn_rl_repo")

STAGE = int(os.environ.get("KSTAGE", "9"))
SUB = int(os.environ.get("KSUB", "9"))

from contextlib import ExitStack

import numpy as np
import ml_dtypes

import concourse.bass as bass
import concourse.bacc as bacc
import concourse.tile as tile
from concourse import mybir
from concourse import bass_utils

B, L, DIN = 4, 8192, 512
H, D, SEG = 8, 64, 512
HD = H * D
NSEG = L // SEG          # 16
NC = 8                   # cores
SPC = NSEG // NC         # segments per core = 2
P = 128
NB = HD // P             # 4 blocks of 128
BS = B * SPC             # batch-segment units per core = 8

bf = mybir.dt.bfloat16
f32 = mybir.dt.float32
AF = mybir.ActivationFunctionType
OP = mybir.AluOpType
bf_np = ml_dtypes.bfloat16

_CACHE = {}


def _build():
    nc = bacc.Bacc(
        "TRN2",
        target_bir_lowering=False,
        debug=False,
        enable_asserts=False,
        num_devices=NC,
    )

    # ---------------- DRAM I/O ----------------
    xt_d = nc.dram_tensor("xt", [B, SPC, NB, P, SEG], bf, kind="ExternalInput").ap()
    wq_d = nc.dram_tensor("wq", [NB, P, HD], bf, kind="ExternalInput").ap()
    wk_d = nc.dram_tensor("wk", [NB, P, HD], bf, kind="ExternalInput").ap()
    wv_d = nc.dram_tensor("wv", [NB, P, HD], bf, kind="ExternalInput").ap()
    wd_d = nc.dram_tensor("wd", [NB, P, D], bf, kind="ExternalInput").ap()
    gcol_d = nc.dram_tensor("gcol", [P, NB], f32, kind="ExternalInput").ap()
    omg_d = nc.dram_tensor("omg", [P, NB], f32, kind="ExternalInput").ap()
    zmask_d = nc.dram_tensor("zmask", [64, NC], f32, kind="ExternalInput").ap()
    oh_d = nc.dram_tensor("oh", [P, NC], f32, kind="ExternalInput").ap()
    mask_d = nc.dram_tensor("cmask", [P, P], bf, kind="ExternalInput").ap()
    ident_d = nc.dram_tensor("ident", [P, P], bf, kind="ExternalInput").ap()
    out_d = nc.dram_tensor("out", [B, SPC, SEG, D], f32, kind="ExternalOutput").ap()

    with tile.TileContext(nc) as tc, ExitStack() as ctx:
        # ---------------- constant / DRAM pools ----------------
        const = ctx.enter_context(tc.tile_pool(name="const", bufs=1))
        dram = ctx.enter_context(tc.tile_pool(name="dram", bufs=1, space="DRAM"))
        keep = ctx.enter_context(tc.tile_pool(name="keep", bufs=BS))
        phb = ctx.enter_context(tc.tile_pool(name="phb", bufs=1))  # phase-B singles

        WQ = const.tile([P, NB, HD], bf)
        WK = const.tile([P, NB, HD], bf)
        WV = const.tile([P, NB, HD], bf)
        WD = const.tile([P, NB, D], bf)
        GC = const.tile([P, NB], f32)
        OMG = const.tile([P, NB], f32)
        ZM = const.tile([64, NC], f32)
        OH = const.tile([P, NC], f32)
        CM = const.tile([P, P], bf)
        ID = const.tile([P, P], bf)
        ONE = const.tile([P, 1], bf)

        nc.sync.dma_start(out=WK, in_=wk_d.rearrange("kb p n -> p kb n"))
        nc.sync.dma_start(out=WV, in_=wv_d.rearrange("kb p n -> p kb n"))
        nc.sync.dma_start(out=WQ, in_=wq_d.rearrange("kb p n -> p kb n"))
        nc.sync.dma_start(out=WD, in_=wd_d.rearrange("kb p n -> p kb n"))
        nc.sync.dma_start(out=GC, in_=gcol_d)
        nc.sync.dma_start(out=OMG, in_=omg_d)
        nc.sync.dma_start(out=ZM, in_=zmask_d)
        nc.sync.dma_start(out=OH, in_=oh_d)
        nc.sync.dma_start(out=CM, in_=mask_d)
        nc.sync.dma_start(out=ID, in_=ident_d)
        nc.vector.memset(ONE, 1.0)

        # collective bounce buffers
        cs_in = dram.tile([BS, HD], f32)
        cs_out = dram.tile([NC * BS, HD], f32, addr_space="Shared")
        ab_in = dram.tile([2, HD, HD], bf)
        zrow_d = dram.tile([BS, HD], bf)
        den_d = dram.tile([BS, H, SEG], bf)
        rca_d = dram.tile([BS, H, SEG], f32)
        rcm_d = dram.tile([BS, SEG], bf)
        rcmr_d = dram.tile([BS, SEG], f32)
        ab_out = dram.tile([NC, 2, HD, HD], bf, addr_space="Shared")

        # retained across phases (bufs=BS -> one slot per batch-segment)
        skT = [keep.tile([P, NB, HD], bf, tag="sk", name=f"sk{i}") for i in range(BS)]
        sqT = [keep.tile([P, NB, SEG], bf, tag="sq", name=f"sq{i}") for i in range(BS)]
        attn_d = dram.tile([BS, NB, P, D], bf)  # attention-part of output

        # z tiles (phase boundary singles)
        ZROW = phb.tile([BS, HD], f32)      # z at segment start, row form
        ZCOL = phb.tile([P, NB, BS], bf)    # column form for denominators
        AT0 = phb.tile([P, NB, HD], bf)     # segment-0 A-part (retained)
        BT0 = phb.tile([P, NB, HD], bf)
        MSEL = phb.tile([P, NB, HD], bf)    # selected M at segment 2c
        MLOC1 = phb.tile([P, NB, HD], bf)   # M at segment 2c+1

        def bs_of(b, j):
            return j * B + b

        # ============ PHASE A + attention (one big pool scope) ============
        with tc.tile_pool(name="pa2", bufs=2) as pa2, \
             tc.tile_pool(name="pva", bufs=BS) as pva, \
             tc.tile_pool(name="pw", bufs=3) as pw, \
             tc.tile_pool(name="pds", bufs=1) as pds, \
             tc.tile_pool(name="pab", bufs=1) as pab, \
             tc.tile_pool(name="ps2", bufs=3, space="PSUM") as ps2, \
             tc.tile_pool(name="psc", bufs=2, space="PSUM") as psc, \
             tc.tile_pool(name="psa", bufs=2, space="PSUM") as psa:
            vaT = [None] * BS
            state = {"at1": None, "bt1": None}

            # ---- A1: k projection + sk + colsums + v projection ----
            a1_ctx = tc.tile_pool(name="ps1c", bufs=1, space="PSUM")
            ps1c = a1_ctx.__enter__()
            for j in range(SPC):
                for b in range(B):
                    i = bs_of(b, j)
                    XT = pa2.tile([P, NB, SEG], bf, tag="xt")
                    nc.sync.dma_start(out=XT, in_=xt_d[b, j].rearrange("kb p s -> p kb s"))
                    sk_i = skT[i]
                    for sb in range(NB):
                        pk = ps2.tile([P, SEG], f32, tag="pp")
                        for kb in range(NB):
                            nc.tensor.matmul(
                                pk,
                                lhsT=XT[:, kb, sb * P:(sb + 1) * P],
                                rhs=WK[:, kb, :],
                                start=(kb == 0),
                                stop=(kb == NB - 1),
                            )
                        # elu1(k) = max(k + 1, exp(min(k, 0)))
                        em = pa2.tile([P, SEG], bf, tag="em")
                        nc.vector.tensor_scalar_min(em, pk, 0.0)
                        ee = pa2.tile([P, SEG], bf, tag="ee")
                        nc.scalar.activation(ee, em, AF.Exp)
                        nc.vector.scalar_tensor_tensor(
                            out=sk_i[:, sb, :], in0=pk, scalar=1.0, in1=ee,
                            op0=OP.add, op1=OP.max,
                        )
                    pc = ps1c.tile([1, HD], f32, tag="pc")
                    for sb in range(NB):
                        nc.tensor.matmul(
                            pc, lhsT=ONE, rhs=sk_i[:, sb, :],
                            start=(sb == 0), stop=(sb == NB - 1),
                        )
                    cs_sb = pa2.tile([1, HD], f32, tag="cs")
                    nc.scalar.activation(cs_sb, pc, AF.Copy)
                    nc.sync.dma_start(out=cs_in[i:i + 1, :], in_=cs_sb)

                    # --- v (original orientation) + aug ones column ---
                    va = pva.tile([P, NB, H, D + 1], bf, tag="va", name=f"va{i}")
                    vaT[i] = va
                    nc.vector.memset(va[:, :, :, D:D + 1], 1.0)
                    for sb in range(NB):
                        pv = ps2.tile([P, SEG], f32, tag="pp")
                        for kb in range(NB):
                            nc.tensor.matmul(
                                pv, lhsT=XT[:, kb, sb * P:(sb + 1) * P],
                                rhs=WV[:, kb, :],
                                start=(kb == 0), stop=(kb == NB - 1),
                            )
                        nc.vector.tensor_copy(
                            va[:, sb, :, 0:D], pv.rearrange("p (h d) -> p h d", h=H)
                        )

            a1_ctx.__exit__(None, None, None)

            # ---- AG1: colsums (hidden behind B_t + unit-0 attention) ----
            if STAGE >= 2:
                nc.gpsimd.collective_compute(
                    "AllGather", OP.bypass,
                    replica_groups=[list(range(NC))],
                    ins=[cs_in.opt()], outs=[cs_out.opt()],
                )

            # ---- B_t for both segments (z-independent) ----
            if SUB >= 4:
                for j in range(SPC):
                    bt_t = pab.tile([P, NB, HD], bf, tag="bt", name=f"bt{j}") if j > 0 else BT0
                    for mb in range(NB):
                        pB = ps2.tile([P, HD], f32, tag="pp")
                        n = 0
                        for b in range(B):
                            for sb in range(NB):
                                nc.tensor.matmul(
                                    pB.rearrange("p (h d) -> p h d", h=H),
                                    lhsT=skT[bs_of(b, j)][:, sb, mb * P:(mb + 1) * P],
                                    rhs=vaT[bs_of(b, j)][:, sb, :, 0:D],
                                    start=(n == 0), stop=(n == B * NB - 1),
                                )
                                n += 1
                        nc.scalar.activation(bt_t[:, mb, :], pB, AF.Copy)
                    if j > 0:
                        state["bt1"] = bt_t

            def unit_qk_attention(b, j):
                """q/kT projections + softmax attention for one unit."""
                i = bs_of(b, j)
                XT = pa2.tile([P, NB, SEG], bf, tag="xt")
                nc.sync.dma_start(out=XT, in_=xt_d[b, j].rearrange("kb p s -> p kb s"))
                va = vaT[i]

                # --- qT (transposed: hd on partitions) ---
                qh = pa2.tile([P, NB, SEG], bf, tag="qh")
                sq_i = sqT[i]
                for mb in range(NB):
                    pq = ps2.tile([P, SEG], f32, tag="pp")
                    for kb in range(NB):
                        nc.tensor.matmul(
                            pq, lhsT=WQ[:, kb, mb * P:(mb + 1) * P],
                            rhs=XT[:, kb, :],
                            start=(kb == 0), stop=(kb == NB - 1),
                        )
                    nc.vector.tensor_copy(qh[:, mb, :], pq)
                    em = pa2.tile([P, SEG], bf, tag="em")
                    nc.vector.tensor_scalar_min(em, pq, 0.0)
                    ee = pa2.tile([P, SEG], bf, tag="ee")
                    nc.scalar.activation(ee, em, AF.Exp)
                    nc.vector.scalar_tensor_tensor(
                        out=sq_i[:, mb, :], in0=pq, scalar=1.0, in1=ee,
                        op0=OP.add, op1=OP.max,
                    )
                # --- kT ---
                kh = pa2.tile([P, NB, SEG], bf, tag="kh", bufs=1)
                for mb in range(NB):
                    pkt = ps2.tile([P, SEG], f32, tag="pp")
                    for kb in range(NB):
                        nc.tensor.matmul(
                            pkt, lhsT=WK[:, kb, mb * P:(mb + 1) * P],
                            rhs=XT[:, kb, :],
                            start=(kb == 0), stop=(kb == NB - 1),
                        )
                    nc.vector.tensor_copy(kh[:, mb, :], pkt)

                if SUB < 2:
                    return
                # --- attention, software-pipelined one head deep: the
                # score/exp/mask chain of head h runs while the value
                # matmuls of head h-1 accumulate, so PE never waits on
                # the exp->mask handoff. ---
                st_i = pa2.tile([P, NB, SEG], bf, tag="stp", name=f"stp{i}")
                dstg = pds.tile([D + 1, H, SEG], bf, tag="dstg")
                wts = {}
                pats = {}
                for h in range(H + 1):
                    if h < H:
                        hb, ho = h // 2, (h % 2) * 64
                        pats[h] = psa.tile([D + 1, SEG], f32, tag="at", name="pat")
                        wtl = []
                        for kb in range(NB):
                            q0 = kb * P
                            qf = SEG - q0
                            ps_ = psc.tile([P, SEG], f32, tag="sc")
                            nc.tensor.matmul(
                                ps_[:, 0:qf],
                                lhsT=kh[ho:ho + 64, hb, q0:q0 + P],
                                rhs=qh[ho:ho + 64, hb, q0:SEG],
                                start=True, stop=True,
                            )
                            wt = pw.tile([P, SEG], bf, tag="wt", bufs=8)
                            nc.scalar.activation(wt[:, 0:qf], ps_[:, 0:qf], AF.Exp,
                                                 scale=0.125)
                            # causal mask on the diagonal 128x128 block
                            nc.vector.tensor_mul(wt[:, 0:P], wt[:, 0:P], CM)
                            wtl.append(wt)
                        wts[h] = wtl
                    if h > 0:
                        hp = h - 1
                        hbp, hop = hp // 2, (hp % 2) * 64
                        pat = pats.pop(hp)
                        wtl = wts.pop(hp)
                        for kb in range(NB):
                            q0 = kb * P
                            qf = SEG - q0
                            nc.tensor.matmul(
                                pat[:, q0:SEG],
                                lhsT=va[:, kb, hp, :],
                                rhs=wtl[kb][:, 0:qf],
                                start=(kb == 0), stop=(kb == NB - 1),
                            )
                        nc.vector.tensor_copy(st_i[hop:hop + 64, hbp, :], pat[0:D, :])
                        nc.scalar.activation(
                            dstg[D:D + 1, hp, :], pat[D:D + 1, :], AF.Copy)
                nc.sync.dma_start(out=den_d[i], in_=dstg[D:D + 1, :, :])
                drs = pw.tile([P, 32], bf, tag="drs")
                nc.sync.dma_start(
                    out=drs,
                    in_=den_d[i].rearrange("h (a f) -> (h a) f", f=32))
                rrs = pw.tile([P, 32], f32, tag="rrs")
                nc.vector.reciprocal(rrs, drs)
                nc.sync.dma_start(
                    out=rca_d[i].rearrange("h (a f) -> (h a) f", f=32),
                    in_=rrs)
                for hb in range(NB):
                    rc2 = pw.tile([P, SEG], f32, tag="rcab", bufs=2)
                    nc.sync.dma_start(
                        out=rc2[0:64, :],
                        in_=rca_d[i:i + 1, 2 * hb, :].partition_broadcast(64))
                    nc.sync.dma_start(
                        out=rc2[64:128, :],
                        in_=rca_d[i:i + 1, 2 * hb + 1, :].partition_broadcast(64))
                    nc.vector.scalar_tensor_tensor(
                        out=st_i[:, hb, :], in0=st_i[:, hb, :],
                        scalar=OMG[:, hb:hb + 1], in1=rc2,
                        op0=OP.mult, op1=OP.mult,
                    )
                # fold Wd into the attention part now: phase B then only
                # adds the memread part (no 4MB st round trip)
                for sb in range(NB):
                    po_a = ps2.tile([P, D], f32, tag="pp", name="poa")
                    for mb in range(NB):
                        nc.tensor.matmul(
                            po_a, lhsT=st_i[:, mb, sb * P:(sb + 1) * P],
                            rhs=WD[:, mb, :],
                            start=(mb == 0), stop=(mb == NB - 1),
                        )
                    ob_a = pw.tile([P, D], bf, tag="oba")
                    nc.vector.tensor_copy(ob_a, po_a)
                    nc.sync.dma_start(out=attn_d[i, sb], in_=ob_a)

            # ---- unit 0 attention first (covers AG1 latency) ----
            unit_qk_attention(0, 0)

            # ---- z prefix, d, in-place rsqrt scale, A_t, composition ----
            if SUB >= 3:
                with tc.tile_pool(name="pz", bufs=1) as pz, \
                     tc.tile_pool(name="psz", bufs=1, space="PSUM") as psz:
                    Z = pz.tile([NC * BS, HD], f32, tag="z")
                    nc.sync.dma_start(out=Z, in_=cs_out)
                    zp = psz.tile([BS, HD], f32, tag="zp", name="zp")
                    nc.tensor.matmul(zp, lhsT=ZM, rhs=Z, start=True, stop=True)
                    nc.scalar.activation(ZROW, zp, AF.Copy, bias=1.0 / D)
                    ZROW16 = pz.tile([BS, HD], bf, tag="zr16")
                    nc.vector.tensor_copy(ZROW16, ZROW)
                    nc.sync.dma_start(out=zrow_d, in_=ZROW16)
                    for kb in range(NB):
                        zc = psz.tile([P, BS], f32, tag="zp", name="zc")
                        nc.tensor.matmul(zc, lhsT=Z[:, kb * P:(kb + 1) * P], rhs=ZM,
                                         start=True, stop=True)
                        nc.scalar.activation(ZCOL[:, kb, :], zc, AF.Copy, bias=1.0 / D)

                    for j in range(SPC):
                        for b in range(B):
                            i = bs_of(b, j)
                            sk_i = skT[i]
                            dcol = pa2.tile([P, NB], f32, tag="d")
                            rcd = pa2.tile([P, NB], f32, tag="rcd")
                            jnk = pa2.tile([P, HD], bf, tag="jnk", bufs=1)
                            zbp = pa2.tile([P, HD], bf, tag="zbp")
                            nc.sync.dma_start(
                                out=zbp,
                                in_=zrow_d[i:i + 1, :].partition_broadcast(P))
                            for sb in range(NB):
                                nc.vector.scalar_tensor_tensor(
                                    out=jnk, in0=sk_i[:, sb, :], scalar=1.0,
                                    in1=zbp, op0=OP.mult, op1=OP.mult,
                                    accum_out=dcol[:, sb:sb + 1],
                                )
                            # scale sk in place by rsqrt(d): A_t is then
                            # just -(s*sk)^T (s*sk), no extra skd tiles
                            nc.vector.reciprocal(rcd, dcol)
                            nc.scalar.activation(rcd, rcd, AF.Sqrt)
                            for sb in range(NB):
                                nc.vector.tensor_scalar_mul(
                                    sk_i[:, sb, :], sk_i[:, sb, :], rcd[:, sb:sb + 1]
                                )

                        if SUB >= 4:
                            # --- A_t (sum over batches) ---
                            at_t = pab.tile([P, NB, HD], bf, tag="at", name=f"at{j}") if j > 0 else AT0
                            for mb in range(NB):
                                pA = ps2.tile([P, HD], f32, tag="pp")
                                n = 0
                                for b in range(B):
                                    for sb in range(NB):
                                        nc.tensor.matmul(
                                            pA,
                                            lhsT=skT[bs_of(b, j)][:, sb, mb * P:(mb + 1) * P],
                                            rhs=skT[bs_of(b, j)][:, sb, :],
                                            start=(n == 0), stop=(n == B * NB - 1),
                                        )
                                        n += 1
                                # negate: A-part = -K
                                nc.scalar.activation(at_t[:, mb, :], pA, AF.Copy, scale=-1.0)
                            if j > 0:
                                state["at1"] = at_t

            if SUB >= 5:
                at1 = state["at1"]
                bt1 = state["bt1"]
                # --- pair composition: abA = Abar^T = A0 A1 + A0 + A1 ---
                abA = pab.tile([P, NB, HD], bf, tag="abA")
                abB = pab.tile([P, NB, HD], bf, tag="abB")
                for mb in range(NB):
                    pA = ps2.tile([P, HD], f32, tag="pp")
                    for kb in range(NB):
                        nc.tensor.matmul(
                            pA, lhsT=AT0[:, kb, mb * P:(mb + 1) * P], rhs=at1[:, kb, :],
                            start=(kb == 0), stop=False,
                        )
                    nc.tensor.matmul(pA, lhsT=ID, rhs=AT0[:, mb, :], start=False, stop=False)
                    nc.tensor.matmul(pA, lhsT=ID, rhs=at1[:, mb, :], start=False, stop=True)
                    nc.scalar.activation(abA[:, mb, :], pA, AF.Copy)
                for mb in range(NB):
                    pB = ps2.tile([P, HD], f32, tag="pp")
                    for kb in range(NB):
                        nc.tensor.matmul(
                            pB, lhsT=at1[:, kb, mb * P:(mb + 1) * P], rhs=BT0[:, kb, :],
                            start=(kb == 0), stop=False,
                        )
                    nc.tensor.matmul(pB, lhsT=ID, rhs=BT0[:, mb, :], start=False, stop=False)
                    nc.tensor.matmul(pB, lhsT=ID, rhs=bt1[:, mb, :], start=False, stop=True)
                    nc.scalar.activation(abB[:, mb, :], pB, AF.Copy)
                nc.sync.dma_start(out=ab_in[0].rearrange("(kb p) n -> p kb n", p=P), in_=abA)
                nc.sync.dma_start(out=ab_in[1].rearrange("(kb p) n -> p kb n", p=P), in_=abB)

            # ---- AG2: pair compositions (flies behind attention) ----
            if STAGE >= 5:
                nc.gpsimd.collective_compute(
                    "AllGather", OP.bypass,
                    replica_groups=[list(range(NC))],
                    ins=[ab_in.opt()], outs=[ab_out.opt()],
                )

            # ---- q/kT + attention for the remaining units ----
            for j in range(SPC):
                for b in range(B):
                    if (b, j) != (0, 0):
                        unit_qk_attention(b, j)

        if STAGE >= 6:
            # ============ phase B prep (independent of AG2), then chain,
            # then memread + combine + Wd ============
            nc.vector.memset(MSEL, 0.0)
            with tc.tile_pool(name="pst", bufs=BS) as pst, \
                 tc.tile_pool(name="pb", bufs=2) as pb, \
                 tc.tile_pool(name="pch", bufs=2) as pch, \
                 tc.tile_pool(name="psch", bufs=NB, space="PSUM") as psch, \
                 tc.tile_pool(name="psb", bufs=2, space="PSUM") as psb, \
                 tc.tile_pool(name="psw", bufs=2, space="PSUM") as psw:

                # ---- prep: load attention terms, memread denominators ----
                # (denominators staged to DRAM, one batched [128,32] exact
                # reciprocal, then per-unit partition broadcasts; st loads
                # ride the scalar queue so they start as soon as each unit's
                # attention lands in DRAM)
                rcmB = [None] * BS
                aoT = [[None] * NB for _ in range(BS)]
                mstg = pb.tile([1, BS, SEG], bf, tag="mstg", bufs=1)
                for j in range(SPC):
                    for b in range(B):
                        i = bs_of(b, j)
                        for sb in range(NB):
                            ao = pst.tile([P, D], bf, tag="ao", name=f"ao{i}_{sb}")
                            aoT[i][sb] = ao
                            nc.sync.dma_start(out=ao, in_=attn_d[i, sb])
                        sq_i = sqT[i]
                        pd = psw.tile([1, SEG], f32, tag="dn", name="pd")
                        for kb in range(NB):
                            nc.tensor.matmul(
                                pd, lhsT=ZCOL[:, kb, i:i + 1], rhs=sq_i[:, kb, :],
                                start=(kb == 0), stop=(kb == NB - 1),
                            )
                        nc.scalar.activation(mstg[0:1, i, :], pd, AF.Copy)
                nc.sync.dma_start(out=rcm_d, in_=mstg[0:1, :, :])
                mrs = pb.tile([P, 32], bf, tag="mrs", bufs=1)
                nc.sync.dma_start(
                    out=mrs, in_=rcm_d.rearrange("i (a f) -> (i a) f", f=32))
                mrr = pb.tile([P, 32], f32, tag="mrr", bufs=1)
                nc.vector.reciprocal(mrr, mrs)
                nc.sync.dma_start(
                    out=rcmr_d.rearrange("i (a f) -> (i a) f", f=32), in_=mrr)
                for i in range(BS):
                    rcmb = pst.tile([P, SEG], f32, tag="rcmb", name=f"rcmb{i}")
                    rcmB[i] = rcmb
                    nc.gpsimd.dma_start(
                        out=rcmb,
                        in_=rcmr_d[i:i + 1, :].partition_broadcast(P))

                # ---- chain + select (cA/cB loads ride the gpsimd queue,
                # which is parked behind AG2 anyway) ----
                pM = [psch.tile([P, HD], f32, tag="ch", name=f"chain{i}") for i in range(NB)]
                mprev = None
                for step in range(NC - 1):
                    cA = pch.tile([P, NB, HD], bf, tag="cA")
                    cB = pch.tile([P, NB, HD], bf, tag="cB")
                    nc.gpsimd.dma_start(
                        out=cA, in_=ab_out[step, 0].rearrange("(kb p) n -> p kb n", p=P))
                    nc.gpsimd.dma_start(
                        out=cB, in_=ab_out[step, 1].rearrange("(kb p) n -> p kb n", p=P))
                    mcur = pch.tile([P, NB, HD], bf, tag="mc")
                    for mb in range(NB):
                        if step == 0:
                            nc.tensor.matmul(pM[mb], lhsT=ID, rhs=cB[:, mb, :],
                                             start=True, stop=True)
                        else:
                            for kb in range(NB):
                                nc.tensor.matmul(
                                    pM[mb], lhsT=cA[:, kb, mb * P:(mb + 1) * P],
                                    rhs=mprev[:, kb, :],
                                    start=False, stop=False,
                                )
                            nc.tensor.matmul(pM[mb], lhsT=ID, rhs=cB[:, mb, :],
                                             start=False, stop=True)
                        nc.scalar.activation(mcur[:, mb, :], pM[mb], AF.Copy)
                        nc.vector.scalar_tensor_tensor(
                            out=MSEL[:, mb, :], in0=mcur[:, mb, :],
                            scalar=OH[:, step:step + 1], in1=MSEL[:, mb, :],
                            op0=OP.mult, op1=OP.add,
                        )
                    mprev = mcur

                # M at segment 2c+1 = M + A0-part @ M + B0
                for mb in range(NB):
                    pm = psb.tile([P, HD], f32, tag="mm")
                    for kb in range(NB):
                        nc.tensor.matmul(
                            pm, lhsT=AT0[:, kb, mb * P:(mb + 1) * P], rhs=MSEL[:, kb, :],
                            start=(kb == 0), stop=False,
                        )
                    nc.tensor.matmul(pm, lhsT=ID, rhs=MSEL[:, mb, :], start=False, stop=False)
                    nc.tensor.matmul(pm, lhsT=ID, rhs=BT0[:, mb, :], start=False, stop=True)
                    nc.scalar.activation(MLOC1[:, mb, :], pm, AF.Copy)

                # ---- memread, combine, Wd ----
                for j in range(SPC):
                    Mt = MSEL if j == 0 else MLOC1
                    for b in range(B):
                        i = bs_of(b, j)
                        sq_i = sqT[i]
                        rcmb = rcmB[i]
                        mts = []
                        for mb in range(NB):
                            pm = psb.tile([P, SEG], f32, tag="mm")
                            for kb in range(NB):
                                nc.tensor.matmul(
                                    pm, lhsT=Mt[:, kb, mb * P:(mb + 1) * P],
                                    rhs=sq_i[:, kb, :],
                                    start=(kb == 0), stop=(kb == NB - 1),
                                )
                            mtmp = pb.tile([P, SEG], bf, tag="mt", bufs=2 * NB)
                            nc.vector.scalar_tensor_tensor(
                                out=mtmp, in0=pm, scalar=GC[:, mb:mb + 1],
                                in1=rcmb,
                                op0=OP.mult, op1=OP.mult,
                            )
                            mts.append(mtmp)
                        for sb in range(NB):
                            po = psw.tile([P, D], f32, tag="dn", name="po")
                            for mb in range(NB):
                                nc.tensor.matmul(
                                    po, lhsT=mts[mb][:, sb * P:(sb + 1) * P],
                                    rhs=WD[:, mb, :],
                                    start=(mb == 0), stop=(mb == NB - 1),
                                )
                            ob = pb.tile([P, D], f32, tag="ob")
                            nc.vector.tensor_add(ob, po, aoT[i][sb])
                            nc.sync.dma_start(
                                out=out_d[b, j, sb * P:(sb + 1) * P, :], in_=ob)

    nc.compile()
    return nc


def _prep_inputs(x, Wq, Wk, Wv, Wd, beta):
    """Host-side prep: transpose/cast/shard. Returns in_maps (list of 8 dicts)."""
    g = 1.0 / (1.0 + np.exp(-beta.astype(np.float64)))  # (H,)
    g = g.astype(np.float32)
    gcol = np.repeat(g, D).reshape(NB, P).T.copy()      # (P, NB): g[(kb*128+p)//64]
    omg = (1.0 - np.repeat(g, D)).reshape(NB, P).T.copy()

    def wprep(w):
        return np.ascontiguousarray(
            w.reshape(NB, P, w.shape[1]).astype(bf_np))

    wq_a, wk_a, wv_a = wprep(Wq), wprep(Wk), wprep(Wv)
    wd_a = wprep(Wd)
    cmask = np.triu(np.ones((P, P), np.float32)).astype(bf_np)
    ident = np.eye(P, dtype=np.float32).astype(bf_np)

    # x -> per-core transposed blocks: xt[b, j, kb, p, s] = x[b, (2c+j)*SEG+s, kb*P+p]
    xs = x.reshape(B, NSEG, SEG, DIN)
    in_maps = []
    for c in range(NC):
        xloc = xs[:, 2 * c:2 * c + 2]                        # (B, SPC, SEG, DIN)
        xt = xloc.transpose(0, 1, 3, 2)                      # (B, SPC, DIN, SEG)
        xt = np.ascontiguousarray(
            xt.reshape(B, SPC, NB, P, SEG).astype(bf_np))
        # AG1 global row for (t, b): rank t//2 contributes row (t%2)*B + b
        zmask = np.zeros((64, NC), np.float32)
        for jj in range(NC):
            tgt = 2 * c + (jj // B)
            bb = jj % B
            for t in range(NSEG):
                if t < tgt:
                    zmask[(t // 2) * BS + (t % 2) * B + bb, jj] = 1.0
        oh = np.zeros((P, NC), np.float32)
        if c >= 1:
            oh[:, c - 1] = 1.0
        in_maps.append({
            "xt": xt, "wq": wq_a, "wk": wk_a, "wv": wv_a, "wd": wd_a,
            "gcol": gcol, "omg": omg, "zmask": zmask, "oh": oh,
            "cmask": cmask, "ident": ident,
        })
    return in_maps


def kernel(x, Wq, Wk, Wv, Wd, beta, _trace=False):
    x = np.asarray(x, np.float32)
    in_maps = _prep_inputs(
        x, np.asarray(Wq, np.float32), np.asarray(Wk, np.float32),
        np.asarray(Wv, np.float32), np.asarray(Wd, np.float32),
        np.asarray(beta, np.float32))
    if "nc" not in _CACHE:
        _CACHE["nc"] = _build()
    nc = _CACHE["nc"]
    res = bass_utils.run_bass_kernel_spmd(
        nc, in_maps, core_ids=list(range(NC)), trace=_trace)
    _CACHE["last_results"] = res
    out = np.empty((B, L, D), np.float32)
    for c in range(NC):
        oc = res.results[c]["out"]                  # (B, SPC, SEG, D)
        out[:, 2 * c * SEG:(2 * c + 2) * SEG, :] = oc.reshape(B, SPC * SEG, D)
    return out


# revision 25
# speedup vs baseline: 1.0325x; 1.0325x over previous
"""Trainium2 Bass kernel for nn_MMHA_78039555768536.

Gated mix of per-segment causal softmax attention and a linear-attention
memory (delta rule, memory summed over batch per segment).

Strategy (8 cores): reformulate the memory recurrence as a linear matrix
recurrence  M_{t+1} = A_t M_t + B_t  with
    A_t = -(s*sk)^T (s*sk)  (A-part; s = rsqrt(d), scaled in place)
    B_t = sum_b sk_b^T v_b
    d_b = sk_b @ z_{b,t};  z is a prefix of column-sums of sk (M-independent)
Core c owns segments {2c, 2c+1} for all batches.  Two all-gathers:
 AG1: per-segment colsums of sk (for the z prefix)  [tiny]
 AG2: per-core pair composition (Abar^T, Bbar)      [1 MB bf16 per rank]
Then every core redundantly runs the 7-step pair chain and selects its own
prefix M via a per-core one-hot input (SPMD, no branches).

Perf structure (phase order chosen so the collectives hide behind
z-independent compute):
  A:   k+v projections, sk, colsums          -> AG1 triggers
  B_t: both segments (z-free)   + unit-0 q/kT/attention   [covers AG1]
  z prefix, d, in-place rsqrt(d) scale, A_t, pair compose -> AG2 triggers
  q/kT projections + attention for units 1..7             [covers AG2]
  phase-B prep (attention loads, memread denominators), 7-step chain,
  memread + combine + Wd.
Softmax/memread reciprocals are batched through DRAM into [128,32] tiles
(DVE reciprocal cost is free-size-bound: ~3.4us for [1,512] vs ~0.25us).
Attention is software-pipelined one head deep: scores/exp/mask of head h
overlap the attention-value matmuls of head h-1, keeping PE dense.
"""

import os
import sys

sys.path.insert(0, "/opt/trn_rl_repo")

STAGE = int(os.environ.get("KSTAGE", "9"))
SUB = int(os.environ.get("KSUB", "9"))

from contextlib import ExitStack

import numpy as np
import ml_dtypes

import concourse.bass as bass
import concourse.bacc as bacc
import concourse.tile as tile
from concourse import mybir
from concourse import bass_utils

B, L, DIN = 4, 8192, 512
H, D, SEG = 8, 64, 512
HD = H * D
NSEG = L // SEG          # 16
NC = 8                   # cores
SPC = NSEG // NC         # segments per core = 2
P = 128
NB = HD // P             # 4 blocks of 128
BS = B * SPC             # batch-segment units per core = 8

bf = mybir.dt.bfloat16
f32 = mybir.dt.float32
AF = mybir.ActivationFunctionType
OP = mybir.AluOpType
bf_np = ml_dtypes.bfloat16

_CACHE = {}


def _build():
    nc = bacc.Bacc(
        "TRN2",
        target_bir_lowering=False,
        debug=False,
        enable_asserts=False,
        num_devices=NC,
    )

    # ---------------- DRAM I/O ----------------
    xt_d = nc.dram_tensor("xt", [B, SPC, NB, P, SEG], bf, kind="ExternalInput").ap()
    wq_d = nc.dram_tensor("wq", [NB, P, HD], bf, kind="ExternalInput").ap()
    wk_d = nc.dram_tensor("wk", [NB, P, HD], bf, kind="ExternalInput").ap()
    wv_d = nc.dram_tensor("wv", [NB, P, HD], bf, kind="ExternalInput").ap()
    wd_d = nc.dram_tensor("wd", [NB, P, D], bf, kind="ExternalInput").ap()
    gcol_d = nc.dram_tensor("gcol", [P, NB], f32, kind="ExternalInput").ap()
    omg_d = nc.dram_tensor("omg", [P, NB], f32, kind="ExternalInput").ap()
    zmask_d = nc.dram_tensor("zmask", [64, NC], f32, kind="ExternalInput").ap()
    oh_d = nc.dram_tensor("oh", [P, NC], f32, kind="ExternalInput").ap()
    mask_d = nc.dram_tensor("cmask", [P, P], bf, kind="ExternalInput").ap()
    ident_d = nc.dram_tensor("ident", [P, P], bf, kind="ExternalInput").ap()
    out_d = nc.dram_tensor("out", [B, SPC, SEG, D], f32, kind="ExternalOutput").ap()

    with tile.TileContext(nc) as tc, ExitStack() as ctx:
        # ---------------- constant / DRAM pools ----------------
        const = ctx.enter_context(tc.tile_pool(name="const", bufs=1))
        dram = ctx.enter_context(tc.tile_pool(name="dram", bufs=1, space="DRAM"))
        keep = ctx.enter_context(tc.tile_pool(name="keep", bufs=BS))
        phb = ctx.enter_context(tc.tile_pool(name="phb", bufs=1))  # phase-B singles

        WQ = const.tile([P, NB, HD], bf)
        WK = const.tile([P, NB, HD], bf)
        WV = const.tile([P, NB, HD], bf)
        WD = const.tile([P, NB, D], bf)
        GC = const.tile([P, NB], f32)
        OMG = const.tile([P, NB], f32)
        ZM = const.tile([64, NC], f32)
        OH = const.tile([P, NC], f32)
        CM = const.tile([P, P], bf)
        ID = const.tile([P, P], bf)
        ONE = const.tile([P, 1], bf)

        nc.sync.dma_start(out=WK, in_=wk_d.rearrange("kb p n -> p kb n"))
        nc.sync.dma_start(out=WV, in_=wv_d.rearrange("kb p n -> p kb n"))
        nc.sync.dma_start(out=WQ, in_=wq_d.rearrange("kb p n -> p kb n"))
        nc.sync.dma_start(out=WD, in_=wd_d.rearrange("kb p n -> p kb n"))
        nc.sync.dma_start(out=GC, in_=gcol_d)
        nc.sync.dma_start(out=OMG, in_=omg_d)
        nc.sync.dma_start(out=ZM, in_=zmask_d)
        nc.sync.dma_start(out=OH, in_=oh_d)
        nc.sync.dma_start(out=CM, in_=mask_d)
        nc.sync.dma_start(out=ID, in_=ident_d)
        nc.vector.memset(ONE, 1.0)

        # collective bounce buffers
        cs_in = dram.tile([BS, HD], f32)
        cs_out = dram.tile([NC * BS, HD], f32, addr_space="Shared")
        ab_in = dram.tile([2, HD, HD], bf)
        zrow_d = dram.tile([BS, HD], bf)
        den_d = dram.tile([BS, H, SEG], bf)
        rca_d = dram.tile([BS, H, SEG], f32)
        rcm_d = dram.tile([BS, SEG], bf)
        rcmr_d = dram.tile([BS, SEG], f32)
        ab_out = dram.tile([NC, 2, HD, HD], bf, addr_space="Shared")

        # retained across phases (bufs=BS -> one slot per batch-segment)
        skT = [keep.tile([P, NB, HD], bf, tag="sk", name=f"sk{i}") for i in range(BS)]
        sqT = [keep.tile([P, NB, SEG], bf, tag="sq", name=f"sq{i}") for i in range(BS)]
        attn_d = dram.tile([BS, NB, P, D], bf)  # attention-part of output

        # z tiles (phase boundary singles)
        ZROW = phb.tile([BS, HD], f32)      # z at segment start, row form
        ZCOL = phb.tile([P, NB, BS], bf)    # column form for denominators
        AT0 = phb.tile([P, NB, HD], bf)     # segment-0 A-part (retained)
        BT0 = phb.tile([P, NB, HD], bf)
        MSEL = phb.tile([P, NB, HD], bf)    # selected M at segment 2c
        MLOC1 = phb.tile([P, NB, HD], bf)   # M at segment 2c+1

        def bs_of(b, j):
            return j * B + b

        # ============ PHASE A + attention (one big pool scope) ============
        with tc.tile_pool(name="pa2", bufs=2) as pa2, \
             tc.tile_pool(name="pva", bufs=BS) as pva, \
             tc.tile_pool(name="pw", bufs=3) as pw, \
             tc.tile_pool(name="pds", bufs=1) as pds, \
             tc.tile_pool(name="pab", bufs=1) as pab, \
             tc.tile_pool(name="ps2", bufs=3, space="PSUM") as ps2, \
             tc.tile_pool(name="psc", bufs=2, space="PSUM") as psc, \
             tc.tile_pool(name="psa", bufs=2, space="PSUM") as psa:
            vaT = [None] * BS
            state = {"at1": None, "bt1": None}
            pend_st = [None]

            def flush_attn_wd():
                """Wd on the previous unit's attention part (deferred one
                unit so the normalize round trip never stalls PE)."""
                if pend_st[0] is None:
                    return
                i, st_i = pend_st[0]
                pend_st[0] = None
                for sb in range(NB):
                    po_a = ps2.tile([P, D], f32, tag="pp", name="poa")
                    for mb in range(NB):
                        nc.tensor.matmul(
                            po_a, lhsT=st_i[:, mb, sb * P:(sb + 1) * P],
                            rhs=WD[:, mb, :],
                            start=(mb == 0), stop=(mb == NB - 1),
                        )
                    ob_a = pw.tile([P, D], bf, tag="oba")
                    nc.vector.tensor_copy(ob_a, po_a)
                    nc.sync.dma_start(out=attn_d[i, sb], in_=ob_a)

            # ---- A1: k projection + sk + colsums + v projection ----
            a1_ctx = tc.tile_pool(name="ps1c", bufs=1, space="PSUM")
            ps1c = a1_ctx.__enter__()
            for j in range(SPC):
                for b in range(B):
                    i = bs_of(b, j)
                    XT = pa2.tile([P, NB, SEG], bf, tag="xt")
                    nc.sync.dma_start(out=XT, in_=xt_d[b, j].rearrange("kb p s -> p kb s"))
                    sk_i = skT[i]
                    for sb in range(NB):
                        pk = ps2.tile([P, SEG], f32, tag="pp")
                        for kb in range(NB):
                            nc.tensor.matmul(
                                pk,
                                lhsT=XT[:, kb, sb * P:(sb + 1) * P],
                                rhs=WK[:, kb, :],
                                start=(kb == 0),
                                stop=(kb == NB - 1),
                            )
                        # elu1(k) = max(k + 1, exp(min(k, 0)))
                        em = pa2.tile([P, SEG], bf, tag="em")
                        nc.vector.tensor_scalar_min(em, pk, 0.0)
                        ee = pa2.tile([P, SEG], bf, tag="ee")
                        nc.scalar.activation(ee, em, AF.Exp)
                        nc.vector.scalar_tensor_tensor(
                            out=sk_i[:, sb, :], in0=pk, scalar=1.0, in1=ee,
                            op0=OP.add, op1=OP.max,
                        )
                    pc = ps1c.tile([1, HD], f32, tag="pc")
                    for sb in range(NB):
                        nc.tensor.matmul(
                            pc, lhsT=ONE, rhs=sk_i[:, sb, :],
                            start=(sb == 0), stop=(sb == NB - 1),
                        )
                    cs_sb = pa2.tile([1, HD], f32, tag="cs")
                    nc.scalar.activation(cs_sb, pc, AF.Copy)
                    nc.sync.dma_start(out=cs_in[i:i + 1, :], in_=cs_sb)

                    # --- v (original orientation) + aug ones column ---
                    va = pva.tile([P, NB, H, D + 1], bf, tag="va", name=f"va{i}")
                    vaT[i] = va
                    nc.vector.memset(va[:, :, :, D:D + 1], 1.0)
                    for sb in range(NB):
                        pv = ps2.tile([P, SEG], f32, tag="pp")
                        for kb in range(NB):
                            nc.tensor.matmul(
                                pv, lhsT=XT[:, kb, sb * P:(sb + 1) * P],
                                rhs=WV[:, kb, :],
                                start=(kb == 0), stop=(kb == NB - 1),
                            )
                        nc.vector.tensor_copy(
                            va[:, sb, :, 0:D], pv.rearrange("p (h d) -> p h d", h=H)
                        )

            a1_ctx.__exit__(None, None, None)

            # ---- AG1: colsums (hidden behind B_t + unit-0 attention) ----
            if STAGE >= 2:
                nc.gpsimd.collective_compute(
                    "AllGather", OP.bypass,
                    replica_groups=[list(range(NC))],
                    ins=[cs_in.opt()], outs=[cs_out.opt()],
                )

            # ---- B_t for both segments (z-independent) ----
            if SUB >= 4:
                for j in range(SPC):
                    bt_t = pab.tile([P, NB, HD], bf, tag="bt", name=f"bt{j}") if j > 0 else BT0
                    for mb in range(NB):
                        pB = ps2.tile([P, HD], f32, tag="pp")
                        n = 0
                        for b in range(B):
                            for sb in range(NB):
                                nc.tensor.matmul(
                                    pB.rearrange("p (h d) -> p h d", h=H),
                                    lhsT=skT[bs_of(b, j)][:, sb, mb * P:(mb + 1) * P],
                                    rhs=vaT[bs_of(b, j)][:, sb, :, 0:D],
                                    start=(n == 0), stop=(n == B * NB - 1),
                                )
                                n += 1
                        nc.scalar.activation(bt_t[:, mb, :], pB, AF.Copy)
                    if j > 0:
                        state["bt1"] = bt_t

            def unit_qk_attention(b, j):
                """q/kT projections + softmax attention for one unit."""
                i = bs_of(b, j)
                XT = pa2.tile([P, NB, SEG], bf, tag="xt")
                nc.sync.dma_start(out=XT, in_=xt_d[b, j].rearrange("kb p s -> p kb s"))
                va = vaT[i]

                # --- qT (transposed: hd on partitions) ---
                qh = pa2.tile([P, NB, SEG], bf, tag="qh")
                sq_i = sqT[i]
                for mb in range(NB):
                    pq = ps2.tile([P, SEG], f32, tag="pp")
                    for kb in range(NB):
                        nc.tensor.matmul(
                            pq, lhsT=WQ[:, kb, mb * P:(mb + 1) * P],
                            rhs=XT[:, kb, :],
                            start=(kb == 0), stop=(kb == NB - 1),
                        )
                    nc.vector.tensor_copy(qh[:, mb, :], pq)
                    em = pa2.tile([P, SEG], bf, tag="em")
                    nc.vector.tensor_scalar_min(em, pq, 0.0)
                    ee = pa2.tile([P, SEG], bf, tag="ee")
                    nc.scalar.activation(ee, em, AF.Exp)
                    nc.vector.scalar_tensor_tensor(
                        out=sq_i[:, mb, :], in0=pq, scalar=1.0, in1=ee,
                        op0=OP.add, op1=OP.max,
                    )
                # --- kT ---
                kh = pa2.tile([P, NB, SEG], bf, tag="kh", bufs=1)
                for mb in range(NB):
                    pkt = ps2.tile([P, SEG], f32, tag="pp")
                    for kb in range(NB):
                        nc.tensor.matmul(
                            pkt, lhsT=WK[:, kb, mb * P:(mb + 1) * P],
                            rhs=XT[:, kb, :],
                            start=(kb == 0), stop=(kb == NB - 1),
                        )
                    nc.vector.tensor_copy(kh[:, mb, :], pkt)

                if SUB < 2:
                    return
                # --- attention, software-pipelined one head deep: the
                # score/exp/mask chain of head h runs while the value
                # matmuls of head h-1 accumulate, so PE never waits on
                # the exp->mask handoff. ---
                st_i = pa2.tile([P, NB, SEG], bf, tag="stp", name=f"stp{i}")
                dstg = pds.tile([D + 1, H, SEG], bf, tag="dstg")
                wts = {}
                pats = {}
                for h in range(H + 1):
                    if h < H:
                        hb, ho = h // 2, (h % 2) * 64
                        pats[h] = psa.tile([D + 1, SEG], f32, tag="at", name="pat")
                        wtl = []
                        for kb in range(NB):
                            q0 = kb * P
                            qf = SEG - q0
                            ps_ = psc.tile([P, SEG], f32, tag="sc")
                            nc.tensor.matmul(
                                ps_[:, 0:qf],
                                lhsT=kh[ho:ho + 64, hb, q0:q0 + P],
                                rhs=qh[ho:ho + 64, hb, q0:SEG],
                                start=True, stop=True,
                            )
                            wt = pw.tile([P, SEG], bf, tag="wt", bufs=8)
                            nc.scalar.activation(wt[:, 0:qf], ps_[:, 0:qf], AF.Exp,
                                                 scale=0.125)
                            # causal mask on the diagonal 128x128 block
                            nc.vector.tensor_mul(wt[:, 0:P], wt[:, 0:P], CM)
                            wtl.append(wt)
                        wts[h] = wtl
                    if h > 0:
                        hp = h - 1
                        hbp, hop = hp // 2, (hp % 2) * 64
                        pat = pats.pop(hp)
                        wtl = wts.pop(hp)
                        for kb in range(NB):
                            q0 = kb * P
                            qf = SEG - q0
                            nc.tensor.matmul(
                                pat[:, q0:SEG],
                                lhsT=va[:, kb, hp, :],
                                rhs=wtl[kb][:, 0:qf],
                                start=(kb == 0), stop=(kb == NB - 1),
                            )
                        nc.vector.tensor_copy(st_i[hop:hop + 64, hbp, :], pat[0:D, :])
                        nc.scalar.activation(
                            dstg[D:D + 1, hp, :], pat[D:D + 1, :], AF.Copy)
                nc.sync.dma_start(out=den_d[i], in_=dstg[D:D + 1, :, :])
                drs = pw.tile([P, 32], bf, tag="drs")
                nc.sync.dma_start(
                    out=drs,
                    in_=den_d[i].rearrange("h (a f) -> (h a) f", f=32))
                rrs = pw.tile([P, 32], f32, tag="rrs")
                nc.vector.reciprocal(rrs, drs)
                nc.sync.dma_start(
                    out=rca_d[i].rearrange("h (a f) -> (h a) f", f=32),
                    in_=rrs)
                for hb in range(NB):
                    rc2 = pw.tile([P, SEG], f32, tag="rcab", bufs=2)
                    nc.sync.dma_start(
                        out=rc2[0:64, :],
                        in_=rca_d[i:i + 1, 2 * hb, :].partition_broadcast(64))
                    nc.sync.dma_start(
                        out=rc2[64:128, :],
                        in_=rca_d[i:i + 1, 2 * hb + 1, :].partition_broadcast(64))
                    nc.vector.scalar_tensor_tensor(
                        out=st_i[:, hb, :], in0=st_i[:, hb, :],
                        scalar=OMG[:, hb:hb + 1], in1=rc2,
                        op0=OP.mult, op1=OP.mult,
                    )
                pend_st[0] = (i, st_i)

            # ---- unit 0 attention first (covers AG1 latency) ----
            unit_qk_attention(0, 0)

            # ---- z prefix, d, in-place rsqrt scale, A_t, composition ----
            if SUB >= 3:
                with tc.tile_pool(name="pz", bufs=1) as pz, \
                     tc.tile_pool(name="psz", bufs=1, space="PSUM") as psz:
                    Z = pz.tile([NC * BS, HD], f32, tag="z")
                    nc.sync.dma_start(out=Z, in_=cs_out)
                    zp = psz.tile([BS, HD], f32, tag="zp", name="zp")
                    nc.tensor.matmul(zp, lhsT=ZM, rhs=Z, start=True, stop=True)
                    nc.scalar.activation(ZROW, zp, AF.Copy, bias=1.0 / D)
                    ZROW16 = pz.tile([BS, HD], bf, tag="zr16")
                    nc.vector.tensor_copy(ZROW16, ZROW)
                    nc.sync.dma_start(out=zrow_d, in_=ZROW16)
                    for kb in range(NB):
                        zc = psz.tile([P, BS], f32, tag="zp", name="zc")
                        nc.tensor.matmul(zc, lhsT=Z[:, kb * P:(kb + 1) * P], rhs=ZM,
                                         start=True, stop=True)
                        nc.scalar.activation(ZCOL[:, kb, :], zc, AF.Copy, bias=1.0 / D)

                    for j in range(SPC):
                        for b in range(B):
                            i = bs_of(b, j)
                            sk_i = skT[i]
                            dcol = pa2.tile([P, NB], f32, tag="d")
                            rcd = pa2.tile([P, NB], f32, tag="rcd")
                            jnk = pa2.tile([P, HD], bf, tag="jnk", bufs=1)
                            zbp = pa2.tile([P, HD], bf, tag="zbp")
                            nc.sync.dma_start(
                                out=zbp,
                                in_=zrow_d[i:i + 1, :].partition_broadcast(P))
                            for sb in range(NB):
                                nc.vector.scalar_tensor_tensor(
                                    out=jnk, in0=sk_i[:, sb, :], scalar=1.0,
                                    in1=zbp, op0=OP.mult, op1=OP.mult,
                                    accum_out=dcol[:, sb:sb + 1],
                                )
                            # scale sk in place by rsqrt(d): A_t is then
                            # just -(s*sk)^T (s*sk), no extra skd tiles
                            nc.vector.reciprocal(rcd, dcol)
                            nc.scalar.activation(rcd, rcd, AF.Sqrt)
                            for sb in range(NB):
                                nc.vector.tensor_scalar_mul(
                                    sk_i[:, sb, :], sk_i[:, sb, :], rcd[:, sb:sb + 1]
                                )

                        if SUB >= 4:
                            # --- A_t (sum over batches) ---
                            at_t = pab.tile([P, NB, HD], bf, tag="at", name=f"at{j}") if j > 0 else AT0
                            for mb in range(NB):
                                pA = ps2.tile([P, HD], f32, tag="pp")
                                n = 0
                                for b in range(B):
                                    for sb in range(NB):
                                        nc.tensor.matmul(
                                            pA,
                                            lhsT=skT[bs_of(b, j)][:, sb, mb * P:(mb + 1) * P],
                                            rhs=skT[bs_of(b, j)][:, sb, :],
                                            start=(n == 0), stop=(n == B * NB - 1),
                                        )
                                        n += 1
                                # negate: A-part = -K
                                nc.scalar.activation(at_t[:, mb, :], pA, AF.Copy, scale=-1.0)
                            if j > 0:
                                state["at1"] = at_t

            if SUB >= 5:
                at1 = state["at1"]
                bt1 = state["bt1"]
                # --- pair composition: abA = Abar^T = A0 A1 + A0 + A1 ---
                abA = pab.tile([P, NB, HD], bf, tag="abA")
                abB = pab.tile([P, NB, HD], bf, tag="abB")
                for mb in range(NB):
                    pA = ps2.tile([P, HD], f32, tag="pp")
                    for kb in range(NB):
                        nc.tensor.matmul(
                            pA, lhsT=AT0[:, kb, mb * P:(mb + 1) * P], rhs=at1[:, kb, :],
                            start=(kb == 0), stop=False,
                        )
                    nc.tensor.matmul(pA, lhsT=ID, rhs=AT0[:, mb, :], start=False, stop=False)
                    nc.tensor.matmul(pA, lhsT=ID, rhs=at1[:, mb, :], start=False, stop=True)
                    nc.scalar.activation(abA[:, mb, :], pA, AF.Copy)
                for mb in range(NB):
                    pB = ps2.tile([P, HD], f32, tag="pp")
                    for kb in range(NB):
                        nc.tensor.matmul(
                            pB, lhsT=at1[:, kb, mb * P:(mb + 1) * P], rhs=BT0[:, kb, :],
                            start=(kb == 0), stop=False,
                        )
                    nc.tensor.matmul(pB, lhsT=ID, rhs=BT0[:, mb, :], start=False, stop=False)
                    nc.tensor.matmul(pB, lhsT=ID, rhs=bt1[:, mb, :], start=False, stop=True)
                    nc.scalar.activation(abB[:, mb, :], pB, AF.Copy)
                nc.sync.dma_start(out=ab_in[0].rearrange("(kb p) n -> p kb n", p=P), in_=abA)
                nc.sync.dma_start(out=ab_in[1].rearrange("(kb p) n -> p kb n", p=P), in_=abB)

            # ---- AG2: pair compositions (flies behind attention) ----
            if STAGE >= 5:
                nc.gpsimd.collective_compute(
                    "AllGather", OP.bypass,
                    replica_groups=[list(range(NC))],
                    ins=[ab_in.opt()], outs=[ab_out.opt()],
                )

            # ---- q/kT + attention for the remaining units ----
            for j in range(SPC):
                for b in range(B):
                    if (b, j) != (0, 0):
                        flush_attn_wd()
                        unit_qk_attention(b, j)
            flush_attn_wd()

        if STAGE >= 6:
            # ============ phase B prep (independent of AG2), then chain,
            # then memread + combine + Wd ============
            nc.vector.memset(MSEL, 0.0)
            with tc.tile_pool(name="pst", bufs=BS) as pst, \
                 tc.tile_pool(name="pb", bufs=2) as pb, \
                 tc.tile_pool(name="pch", bufs=2) as pch, \
                 tc.tile_pool(name="psch", bufs=NB, space="PSUM") as psch, \
                 tc.tile_pool(name="psb", bufs=2, space="PSUM") as psb, \
                 tc.tile_pool(name="psw", bufs=2, space="PSUM") as psw:

                # ---- prep: load attention terms, memread denominators ----
                # (denominators staged to DRAM, one batched [128,32] exact
                # reciprocal, then per-unit partition broadcasts; st loads
                # ride the scalar queue so they start as soon as each unit's
                # attention lands in DRAM)
                rcmB = [None] * BS
                aoT = [[None] * NB for _ in range(BS)]
                mstg = pb.tile([1, BS, SEG], bf, tag="mstg", bufs=1)
                for j in range(SPC):
                    for b in range(B):
                        i = bs_of(b, j)
                        for sb in range(NB):
                            ao = pst.tile([P, D], bf, tag="ao", name=f"ao{i}_{sb}")
                            aoT[i][sb] = ao
                            nc.sync.dma_start(out=ao, in_=attn_d[i, sb])
                        sq_i = sqT[i]
                        pd = psw.tile([1, SEG], f32, tag="dn", name="pd")
                        for kb in range(NB):
                            nc.tensor.matmul(
                                pd, lhsT=ZCOL[:, kb, i:i + 1], rhs=sq_i[:, kb, :],
                                start=(kb == 0), stop=(kb == NB - 1),
                            )
                        nc.scalar.activation(mstg[0:1, i, :], pd, AF.Copy)
                nc.sync.dma_start(out=rcm_d, in_=mstg[0:1, :, :])
                mrs = pb.tile([P, 32], bf, tag="mrs", bufs=1)
                nc.sync.dma_start(
                    out=mrs, in_=rcm_d.rearrange("i (a f) -> (i a) f", f=32))
                mrr = pb.tile([P, 32], f32, tag="mrr", bufs=1)
                nc.vector.reciprocal(mrr, mrs)
                nc.sync.dma_start(
                    out=rcmr_d.rearrange("i (a f) -> (i a) f", f=32), in_=mrr)
                for i in range(BS):
                    rcmb = pst.tile([P, SEG], f32, tag="rcmb", name=f"rcmb{i}")
                    rcmB[i] = rcmb
                    nc.gpsimd.dma_start(
                        out=rcmb,
                        in_=rcmr_d[i:i + 1, :].partition_broadcast(P))

                # ---- chain + select (cA/cB loads ride the gpsimd queue,
                # which is parked behind AG2 anyway) ----
                pM = [psch.tile([P, HD], f32, tag="ch", name=f"chain{i}") for i in range(NB)]
                mprev = None
                for step in range(NC - 1):
                    cA = pch.tile([P, NB, HD], bf, tag="cA")
                    cB = pch.tile([P, NB, HD], bf, tag="cB")
                    nc.gpsimd.dma_start(
                        out=cA, in_=ab_out[step, 0].rearrange("(kb p) n -> p kb n", p=P))
                    nc.gpsimd.dma_start(
                        out=cB, in_=ab_out[step, 1].rearrange("(kb p) n -> p kb n", p=P))
                    mcur = pch.tile([P, NB, HD], bf, tag="mc")
                    for mb in range(NB):
                        if step == 0:
                            nc.tensor.matmul(pM[mb], lhsT=ID, rhs=cB[:, mb, :],
                                             start=True, stop=True)
                        else:
                            for kb in range(NB):
                                nc.tensor.matmul(
                                    pM[mb], lhsT=cA[:, kb, mb * P:(mb + 1) * P],
                                    rhs=mprev[:, kb, :],
                                    start=False, stop=False,
                                )
                            nc.tensor.matmul(pM[mb], lhsT=ID, rhs=cB[:, mb, :],
                                             start=False, stop=True)
                        nc.scalar.activation(mcur[:, mb, :], pM[mb], AF.Copy)
                        nc.vector.scalar_tensor_tensor(
                            out=MSEL[:, mb, :], in0=mcur[:, mb, :],
                            scalar=OH[:, step:step + 1], in1=MSEL[:, mb, :],
                            op0=OP.mult, op1=OP.add,
                        )
                    mprev = mcur

                # M at segment 2c+1 = M + A0-part @ M + B0
                for mb in range(NB):
                    pm = psb.tile([P, HD], f32, tag="mm")
                    for kb in range(NB):
                        nc.tensor.matmul(
                            pm, lhsT=AT0[:, kb, mb * P:(mb + 1) * P], rhs=MSEL[:, kb, :],
                            start=(kb == 0), stop=False,
                        )
                    nc.tensor.matmul(pm, lhsT=ID, rhs=MSEL[:, mb, :], start=False, stop=False)
                    nc.tensor.matmul(pm, lhsT=ID, rhs=BT0[:, mb, :], start=False, stop=True)
                    nc.scalar.activation(MLOC1[:, mb, :], pm, AF.Copy)

                # ---- memread, combine, Wd ----
                for j in range(SPC):
                    Mt = MSEL if j == 0 else MLOC1
                    for b in range(B):
                        i = bs_of(b, j)
                        sq_i = sqT[i]
                        rcmb = rcmB[i]
                        mts = []
                        for mb in range(NB):
                            pm = psb.tile([P, SEG], f32, tag="mm")
                            for kb in range(NB):
                                nc.tensor.matmul(
                                    pm, lhsT=Mt[:, kb, mb * P:(mb + 1) * P],
                                    rhs=sq_i[:, kb, :],
                                    start=(kb == 0), stop=(kb == NB - 1),
                                )
                            mtmp = pb.tile([P, SEG], bf, tag="mt", bufs=2 * NB)
                            nc.vector.scalar_tensor_tensor(
                                out=mtmp, in0=pm, scalar=GC[:, mb:mb + 1],
                                in1=rcmb,
                                op0=OP.mult, op1=OP.mult,
                            )
                            mts.append(mtmp)
                        for sb in range(NB):
                            po = psw.tile([P, D], f32, tag="dn", name="po")
                            for mb in range(NB):
                                nc.tensor.matmul(
                                    po, lhsT=mts[mb][:, sb * P:(sb + 1) * P],
                                    rhs=WD[:, mb, :],
                                    start=(mb == 0), stop=(mb == NB - 1),
                                )
                            ob = pb.tile([P, D], f32, tag="ob")
                            nc.vector.tensor_add(ob, po, aoT[i][sb])
                            nc.sync.dma_start(
                                out=out_d[b, j, sb * P:(sb + 1) * P, :], in_=ob)

    nc.compile()
    return nc


def _prep_inputs(x, Wq, Wk, Wv, Wd, beta):
    """Host-side prep: transpose/cast/shard. Returns in_maps (list of 8 dicts)."""
    g = 1.0 / (1.0 + np.exp(-beta.astype(np.float64)))  # (H,)
    g = g.astype(np.float32)
    gcol = np.repeat(g, D).reshape(NB, P).T.copy()      # (P, NB): g[(kb*128+p)//64]
    omg = (1.0 - np.repeat(g, D)).reshape(NB, P).T.copy()

    def wprep(w):
        return np.ascontiguousarray(
            w.reshape(NB, P, w.shape[1]).astype(bf_np))

    wq_a, wk_a, wv_a = wprep(Wq), wprep(Wk), wprep(Wv)
    wd_a = wprep(Wd)
    cmask = np.triu(np.ones((P, P), np.float32)).astype(bf_np)
    ident = np.eye(P, dtype=np.float32).astype(bf_np)

    # x -> per-core transposed blocks: xt[b, j, kb, p, s] = x[b, (2c+j)*SEG+s, kb*P+p]
    xs = x.reshape(B, NSEG, SEG, DIN)
    in_maps = []
    for c in range(NC):
        xloc = xs[:, 2 * c:2 * c + 2]                        # (B, SPC, SEG, DIN)
        xt = xloc.transpose(0, 1, 3, 2)                      # (B, SPC, DIN, SEG)
        xt = np.ascontiguousarray(
            xt.reshape(B, SPC, NB, P, SEG).astype(bf_np))
        # AG1 global row for (t, b): rank t//2 contributes row (t%2)*B + b
        zmask = np.zeros((64, NC), np.float32)
        for jj in range(NC):
            tgt = 2 * c + (jj // B)
            bb = jj % B
            for t in range(NSEG):
                if t < tgt:
                    zmask[(t // 2) * BS + (t % 2) * B + bb, jj] = 1.0
        oh = np.zeros((P, NC), np.float32)
        if c >= 1:
            oh[:, c - 1] = 1.0
        in_maps.append({
            "xt": xt, "wq": wq_a, "wk": wk_a, "wv": wv_a, "wd": wd_a,
            "gcol": gcol, "omg": omg, "zmask": zmask, "oh": oh,
            "cmask": cmask, "ident": ident,
        })
    return in_maps


def kernel(x, Wq, Wk, Wv, Wd, beta, _trace=False):
    x = np.asarray(x, np.float32)
    in_maps = _prep_inputs(
        x, np.asarray(Wq, np.float32), np.asarray(Wk, np.float32),
        np.asarray(Wv, np.float32), np.asarray(Wd, np.float32),
        np.asarray(beta, np.float32))
    if "nc" not in _CACHE:
        _CACHE["nc"] = _build()
    nc = _CACHE["nc"]
    res = bass_utils.run_bass_kernel_spmd(
        nc, in_maps, core_ids=list(range(NC)), trace=_trace)
    _CACHE["last_results"] = res
    out = np.empty((B, L, D), np.float32)
    for c in range(NC):
        oc = res.results[c]["out"]                  # (B, SPC, SEG, D)
        out[:, 2 * c * SEG:(2 * c + 2) * SEG, :] = oc.reshape(B, SPC * SEG, D)
    return out


# revision 26
# speedup vs baseline: 1.2825x; 1.2422x over previous
"""Trainium2 Bass kernel for nn_MMHA_78039555768536.

Gated mix of per-segment causal softmax attention and a linear-attention
memory (delta rule, memory summed over batch per segment).

Strategy (8 cores): reformulate the memory recurrence as a linear matrix
recurrence  M_{t+1} = A_t M_t + B_t  with
    A_t = -(s*sk)^T (s*sk)  (A-part; s = rsqrt(d), scaled in place)
    B_t = sum_b sk_b^T v_b
    d_b = sk_b @ z_{b,t};  z is a prefix of column-sums of sk (M-independent)
Core c owns segments {2c, 2c+1} for all batches.  Two all-gathers:
 AG1: per-segment colsums of sk (for the z prefix)  [tiny]
 AG2: per-core pair composition (Abar^T, Bbar)      [1 MB bf16 per rank]
Then every core redundantly runs the 7-step pair chain and selects its own
prefix M via a per-core one-hot input (SPMD, no branches).

Perf structure (phase order chosen so the collectives hide behind
z-independent compute):
  A:   k+v projections, sk, colsums          -> AG1 triggers
  B_t: both segments (z-free)   + unit-0 q/kT/attention   [covers AG1]
  z prefix, d, in-place rsqrt(d) scale, A_t, pair compose -> AG2 triggers
  q/kT projections + attention for units 1..7             [covers AG2]
  phase-B prep (attention loads, memread denominators), 7-step chain,
  memread + combine + Wd.
Softmax/memread reciprocals are batched through DRAM into [128,32] tiles
(DVE reciprocal cost is free-size-bound: ~3.4us for [1,512] vs ~0.25us).
Attention is software-pipelined one head deep: scores/exp/mask of head h
overlap the attention-value matmuls of head h-1, keeping PE dense.
"""

import os
import sys

sys.path.insert(0, "/opt/trn_rl_repo")

STAGE = int(os.environ.get("KSTAGE", "9"))
SUB = int(os.environ.get("KSUB", "9"))

from contextlib import ExitStack

import numpy as np
import ml_dtypes

import concourse.bass as bass
import concourse.bacc as bacc
import concourse.tile as tile
from concourse import mybir
from concourse import bass_utils

B, L, DIN = 4, 8192, 512
H, D, SEG = 8, 64, 512
HD = H * D
NSEG = L // SEG          # 16
NC = 8                   # cores
SPC = NSEG // NC         # segments per core = 2
P = 128
NB = HD // P             # 4 blocks of 128
BS = B * SPC             # batch-segment units per core = 8

bf = mybir.dt.bfloat16
f32 = mybir.dt.float32
AF = mybir.ActivationFunctionType
OP = mybir.AluOpType
bf_np = ml_dtypes.bfloat16

_CACHE = {}


def _build():
    nc = bacc.Bacc(
        "TRN2",
        target_bir_lowering=False,
        debug=False,
        enable_asserts=False,
        num_devices=NC,
    )

    # ---------------- DRAM I/O ----------------
    xt_d = nc.dram_tensor("xt", [B, SPC, NB, P, SEG], bf, kind="ExternalInput").ap()
    wq_d = nc.dram_tensor("wq", [NB, P, HD], bf, kind="ExternalInput").ap()
    wk_d = nc.dram_tensor("wk", [NB, P, HD], bf, kind="ExternalInput").ap()
    wv_d = nc.dram_tensor("wv", [NB, P, HD], bf, kind="ExternalInput").ap()
    wd_d = nc.dram_tensor("wd", [NB, P, D], bf, kind="ExternalInput").ap()
    gcol_d = nc.dram_tensor("gcol", [P, NB], f32, kind="ExternalInput").ap()
    omg_d = nc.dram_tensor("omg", [P, NB], f32, kind="ExternalInput").ap()
    zmask_d = nc.dram_tensor("zmask", [64, NC], f32, kind="ExternalInput").ap()
    oh_d = nc.dram_tensor("oh", [P, NC], f32, kind="ExternalInput").ap()
    mask_d = nc.dram_tensor("cmask", [P, P], bf, kind="ExternalInput").ap()
    ident_d = nc.dram_tensor("ident", [P, P], bf, kind="ExternalInput").ap()
    out_d = nc.dram_tensor("out", [B, SPC, SEG, D], f32, kind="ExternalOutput").ap()

    with tile.TileContext(nc) as tc, ExitStack() as ctx:
        # ---------------- constant / DRAM pools ----------------
        const = ctx.enter_context(tc.tile_pool(name="const", bufs=1))
        dram = ctx.enter_context(tc.tile_pool(name="dram", bufs=1, space="DRAM"))
        keep = ctx.enter_context(tc.tile_pool(name="keep", bufs=BS))
        phb = ctx.enter_context(tc.tile_pool(name="phb", bufs=1))  # phase-B singles

        WQ = const.tile([P, NB, HD], bf)
        WK = const.tile([P, NB, HD], bf)
        WV = const.tile([P, NB, HD], bf)
        WD = const.tile([P, NB, D], bf)
        GC = const.tile([P, NB], f32)
        OMG = const.tile([P, NB], f32)
        ZM = const.tile([64, NC], f32)
        OH = const.tile([P, NC], f32)
        CM = const.tile([P, P], bf)
        ID = const.tile([P, P], bf)
        ONE = const.tile([P, 1], bf)

        nc.sync.dma_start(out=WK, in_=wk_d.rearrange("kb p n -> p kb n"))
        nc.sync.dma_start(out=WV, in_=wv_d.rearrange("kb p n -> p kb n"))
        nc.sync.dma_start(out=WQ, in_=wq_d.rearrange("kb p n -> p kb n"))
        nc.sync.dma_start(out=WD, in_=wd_d.rearrange("kb p n -> p kb n"))
        nc.sync.dma_start(out=GC, in_=gcol_d)
        nc.sync.dma_start(out=OMG, in_=omg_d)
        nc.sync.dma_start(out=ZM, in_=zmask_d)
        nc.sync.dma_start(out=OH, in_=oh_d)
        nc.sync.dma_start(out=CM, in_=mask_d)
        nc.sync.dma_start(out=ID, in_=ident_d)
        nc.vector.memset(ONE, 1.0)

        # collective bounce buffers
        cs_in = dram.tile([BS, HD], f32)
        cs_out = dram.tile([NC * BS, HD], f32, addr_space="Shared")
        ab_in = dram.tile([2, HD, HD], bf)
        zrow_d = dram.tile([BS, HD], bf)
        den_d = dram.tile([BS, H, SEG], bf)
        rca_d = dram.tile([BS, H, SEG], f32)
        rcm_d = dram.tile([BS, SEG], bf)
        rcmr_d = dram.tile([BS, SEG], f32)
        ab_out = dram.tile([NC, 2, HD, HD], bf, addr_space="Shared")

        # retained across phases (bufs=BS -> one slot per batch-segment)
        skT = [keep.tile([P, NB, HD], bf, tag="sk", name=f"sk{i}") for i in range(BS)]
        sqT = [keep.tile([P, NB, SEG], bf, tag="sq", name=f"sq{i}") for i in range(BS)]
        step_d = dram.tile([BS, NB, P, SEG], bf)  # attention-term scratch

        # z tiles (phase boundary singles)
        ZROW = phb.tile([BS, HD], f32)      # z at segment start, row form
        ZCOL = phb.tile([P, NB, BS], bf)    # column form for denominators
        AT0 = phb.tile([P, NB, HD], bf)     # segment-0 A-part (retained)
        BT0 = phb.tile([P, NB, HD], bf)
        MSEL = phb.tile([P, NB, HD], bf)    # selected M at segment 2c
        MLOC1 = phb.tile([P, NB, HD], bf)   # M at segment 2c+1

        def bs_of(b, j):
            return j * B + b

        # ============ PHASE A + attention (one big pool scope) ============
        with tc.tile_pool(name="pa2", bufs=2) as pa2, \
             tc.tile_pool(name="pva", bufs=BS) as pva, \
             tc.tile_pool(name="pw", bufs=3) as pw, \
             tc.tile_pool(name="pds", bufs=1) as pds, \
             tc.tile_pool(name="pab", bufs=1) as pab, \
             tc.tile_pool(name="ps2", bufs=3, space="PSUM") as ps2, \
             tc.tile_pool(name="psc", bufs=2, space="PSUM") as psc, \
             tc.tile_pool(name="psa", bufs=2, space="PSUM") as psa:
            vaT = [None] * BS
            state = {"at1": None, "bt1": None}

            # ---- A1: k projection + sk + colsums + v projection ----
            a1_ctx = tc.tile_pool(name="ps1c", bufs=1, space="PSUM")
            ps1c = a1_ctx.__enter__()
            for j in range(SPC):
                for b in range(B):
                    i = bs_of(b, j)
                    XT = pa2.tile([P, NB, SEG], bf, tag="xt")
                    nc.sync.dma_start(out=XT, in_=xt_d[b, j].rearrange("kb p s -> p kb s"))
                    sk_i = skT[i]
                    for sb in range(NB):
                        pk = ps2.tile([P, SEG], f32, tag="pp")
                        for kb in range(NB):
                            nc.tensor.matmul(
                                pk,
                                lhsT=XT[:, kb, sb * P:(sb + 1) * P],
                                rhs=WK[:, kb, :],
                                start=(kb == 0),
                                stop=(kb == NB - 1),
                            )
                        # elu1(k) = max(k + 1, exp(min(k, 0)))
                        em = pa2.tile([P, SEG], bf, tag="em")
                        nc.vector.tensor_scalar_min(em, pk, 0.0)
                        ee = pa2.tile([P, SEG], bf, tag="ee")
                        nc.scalar.activation(ee, em, AF.Exp)
                        nc.vector.scalar_tensor_tensor(
                            out=sk_i[:, sb, :], in0=pk, scalar=1.0, in1=ee,
                            op0=OP.add, op1=OP.max,
                        )
                    pc = ps1c.tile([1, HD], f32, tag="pc")
                    for sb in range(NB):
                        nc.tensor.matmul(
                            pc, lhsT=ONE, rhs=sk_i[:, sb, :],
                            start=(sb == 0), stop=(sb == NB - 1),
                        )
                    cs_sb = pa2.tile([1, HD], f32, tag="cs")
                    nc.scalar.activation(cs_sb, pc, AF.Copy)
                    nc.sync.dma_start(out=cs_in[i:i + 1, :], in_=cs_sb)

                    # --- v (original orientation) + aug ones column ---
                    va = pva.tile([P, NB, H, D + 1], bf, tag="va", name=f"va{i}")
                    vaT[i] = va
                    nc.vector.memset(va[:, :, :, D:D + 1], 1.0)
                    for sb in range(NB):
                        pv = ps2.tile([P, SEG], f32, tag="pp")
                        for kb in range(NB):
                            nc.tensor.matmul(
                                pv, lhsT=XT[:, kb, sb * P:(sb + 1) * P],
                                rhs=WV[:, kb, :],
                                start=(kb == 0), stop=(kb == NB - 1),
                            )
                        nc.vector.tensor_copy(
                            va[:, sb, :, 0:D], pv.rearrange("p (h d) -> p h d", h=H)
                        )

            a1_ctx.__exit__(None, None, None)

            # ---- AG1: colsums (hidden behind B_t + unit-0 attention) ----
            if STAGE >= 2:
                nc.gpsimd.collective_compute(
                    "AllGather", OP.bypass,
                    replica_groups=[list(range(NC))],
                    ins=[cs_in.opt()], outs=[cs_out.opt()],
                )

            # ---- B_t for both segments (z-independent) ----
            if SUB >= 4:
                for j in range(SPC):
                    bt_t = pab.tile([P, NB, HD], bf, tag="bt", name=f"bt{j}") if j > 0 else BT0
                    for mb in range(NB):
                        pB = ps2.tile([P, HD], f32, tag="pp")
                        n = 0
                        for b in range(B):
                            for sb in range(NB):
                                nc.tensor.matmul(
                                    pB.rearrange("p (h d) -> p h d", h=H),
                                    lhsT=skT[bs_of(b, j)][:, sb, mb * P:(mb + 1) * P],
                                    rhs=vaT[bs_of(b, j)][:, sb, :, 0:D],
                                    start=(n == 0), stop=(n == B * NB - 1),
                                )
                                n += 1
                        nc.scalar.activation(bt_t[:, mb, :], pB, AF.Copy)
                    if j > 0:
                        state["bt1"] = bt_t

            def unit_qk_attention(b, j):
                """q/kT projections + softmax attention for one unit."""
                i = bs_of(b, j)
                XT = pa2.tile([P, NB, SEG], bf, tag="xt")
                nc.sync.dma_start(out=XT, in_=xt_d[b, j].rearrange("kb p s -> p kb s"))
                va = vaT[i]

                # --- qT (transposed: hd on partitions) ---
                qh = pa2.tile([P, NB, SEG], bf, tag="qh")
                sq_i = sqT[i]
                for mb in range(NB):
                    pq = ps2.tile([P, SEG], f32, tag="pp")
                    for kb in range(NB):
                        nc.tensor.matmul(
                            pq, lhsT=WQ[:, kb, mb * P:(mb + 1) * P],
                            rhs=XT[:, kb, :],
                            start=(kb == 0), stop=(kb == NB - 1),
                        )
                    nc.vector.tensor_copy(qh[:, mb, :], pq)
                    em = pa2.tile([P, SEG], bf, tag="em")
                    nc.vector.tensor_scalar_min(em, pq, 0.0)
                    ee = pa2.tile([P, SEG], bf, tag="ee")
                    nc.scalar.activation(ee, em, AF.Exp)
                    nc.vector.scalar_tensor_tensor(
                        out=sq_i[:, mb, :], in0=pq, scalar=1.0, in1=ee,
                        op0=OP.add, op1=OP.max,
                    )
                # --- kT ---
                kh = pa2.tile([P, NB, SEG], bf, tag="kh", bufs=1)
                for mb in range(NB):
                    pkt = ps2.tile([P, SEG], f32, tag="pp")
                    for kb in range(NB):
                        nc.tensor.matmul(
                            pkt, lhsT=WK[:, kb, mb * P:(mb + 1) * P],
                            rhs=XT[:, kb, :],
                            start=(kb == 0), stop=(kb == NB - 1),
                        )
                    nc.vector.tensor_copy(kh[:, mb, :], pkt)

                if SUB < 2:
                    return
                # --- attention, software-pipelined one head deep: the
                # score/exp/mask chain of head h runs while the value
                # matmuls of head h-1 accumulate, so PE never waits on
                # the exp->mask handoff. ---
                st_i = pa2.tile([P, NB, SEG], bf, tag="stp", name=f"stp{i}")
                dstg = pds.tile([D + 1, H, SEG], bf, tag="dstg")
                wts = {}
                pats = {}
                for h in range(H + 1):
                    if h < H:
                        hb, ho = h // 2, (h % 2) * 64
                        pats[h] = psa.tile([D + 1, SEG], f32, tag="at", name="pat")
                        wtl = []
                        for kb in range(NB):
                            q0 = kb * P
                            qf = SEG - q0
                            ps_ = psc.tile([P, SEG], f32, tag="sc")
                            nc.tensor.matmul(
                                ps_[:, 0:qf],
                                lhsT=kh[ho:ho + 64, hb, q0:q0 + P],
                                rhs=qh[ho:ho + 64, hb, q0:SEG],
                                start=True, stop=True,
                            )
                            wt = pw.tile([P, SEG], bf, tag="wt", bufs=8)
                            nc.scalar.activation(wt[:, 0:qf], ps_[:, 0:qf], AF.Exp,
                                                 scale=0.125)
                            # causal mask on the diagonal 128x128 block
                            nc.vector.tensor_mul(wt[:, 0:P], wt[:, 0:P], CM)
                            wtl.append(wt)
                        wts[h] = wtl
                    if h > 0:
                        hp = h - 1
                        hbp, hop = hp // 2, (hp % 2) * 64
                        pat = pats.pop(hp)
                        wtl = wts.pop(hp)
                        for kb in range(NB):
                            q0 = kb * P
                            qf = SEG - q0
                            nc.tensor.matmul(
                                pat[:, q0:SEG],
                                lhsT=va[:, kb, hp, :],
                                rhs=wtl[kb][:, 0:qf],
                                start=(kb == 0), stop=(kb == NB - 1),
                            )
                        nc.vector.tensor_copy(st_i[hop:hop + 64, hbp, :], pat[0:D, :])
                        nc.scalar.activation(
                            dstg[D:D + 1, hp, :], pat[D:D + 1, :], AF.Copy)
                nc.sync.dma_start(out=den_d[i], in_=dstg[D:D + 1, :, :])
                drs = pw.tile([P, 32], bf, tag="drs")
                nc.sync.dma_start(
                    out=drs,
                    in_=den_d[i].rearrange("h (a f) -> (h a) f", f=32))
                rrs = pw.tile([P, 32], f32, tag="rrs")
                nc.vector.reciprocal(rrs, drs)
                nc.sync.dma_start(
                    out=rca_d[i].rearrange("h (a f) -> (h a) f", f=32),
                    in_=rrs)
                for hb in range(NB):
                    rc2 = pw.tile([P, SEG], f32, tag="rcab", bufs=2)
                    nc.sync.dma_start(
                        out=rc2[0:64, :],
                        in_=rca_d[i:i + 1, 2 * hb, :].partition_broadcast(64))
                    nc.sync.dma_start(
                        out=rc2[64:128, :],
                        in_=rca_d[i:i + 1, 2 * hb + 1, :].partition_broadcast(64))
                    nc.vector.scalar_tensor_tensor(
                        out=st_i[:, hb, :], in0=st_i[:, hb, :],
                        scalar=OMG[:, hb:hb + 1], in1=rc2,
                        op0=OP.mult, op1=OP.mult,
                    )
                nc.sync.dma_start(
                    out=step_d[i].rearrange("kb p s -> p kb s"), in_=st_i)

            # ---- unit 0 attention first (covers AG1 latency) ----
            unit_qk_attention(0, 0)

            # ---- z prefix, d, in-place rsqrt scale, A_t, composition ----
            if SUB >= 3:
                with tc.tile_pool(name="pz", bufs=1) as pz, \
                     tc.tile_pool(name="psz", bufs=1, space="PSUM") as psz:
                    Z = pz.tile([NC * BS, HD], f32, tag="z")
                    nc.sync.dma_start(out=Z, in_=cs_out)
                    zp = psz.tile([BS, HD], f32, tag="zp", name="zp")
                    nc.tensor.matmul(zp, lhsT=ZM, rhs=Z, start=True, stop=True)
                    nc.scalar.activation(ZROW, zp, AF.Copy, bias=1.0 / D)
                    ZROW16 = pz.tile([BS, HD], bf, tag="zr16")
                    nc.vector.tensor_copy(ZROW16, ZROW)
                    nc.sync.dma_start(out=zrow_d, in_=ZROW16)
                    for kb in range(NB):
                        zc = psz.tile([P, BS], f32, tag="zp", name="zc")
                        nc.tensor.matmul(zc, lhsT=Z[:, kb * P:(kb + 1) * P], rhs=ZM,
                                         start=True, stop=True)
                        nc.scalar.activation(ZCOL[:, kb, :], zc, AF.Copy, bias=1.0 / D)

                    for j in range(SPC):
                        for b in range(B):
                            i = bs_of(b, j)
                            sk_i = skT[i]
                            dcol = pa2.tile([P, NB], f32, tag="d")
                            rcd = pa2.tile([P, NB], f32, tag="rcd")
                            jnk = pa2.tile([P, HD], bf, tag="jnk", bufs=1)
                            zbp = pa2.tile([P, HD], bf, tag="zbp")
                            nc.sync.dma_start(
                                out=zbp,
                                in_=zrow_d[i:i + 1, :].partition_broadcast(P))
                            for sb in range(NB):
                                nc.vector.scalar_tensor_tensor(
                                    out=jnk, in0=sk_i[:, sb, :], scalar=1.0,
                                    in1=zbp, op0=OP.mult, op1=OP.mult,
                                    accum_out=dcol[:, sb:sb + 1],
                                )
                            # scale sk in place by rsqrt(d): A_t is then
                            # just -(s*sk)^T (s*sk), no extra skd tiles
                            nc.vector.reciprocal(rcd, dcol)
                            nc.scalar.activation(rcd, rcd, AF.Sqrt)
                            for sb in range(NB):
                                nc.vector.tensor_scalar_mul(
                                    sk_i[:, sb, :], sk_i[:, sb, :], rcd[:, sb:sb + 1]
                                )

                        if SUB >= 4:
                            # --- A_t (sum over batches) ---
                            at_t = pab.tile([P, NB, HD], bf, tag="at", name=f"at{j}") if j > 0 else AT0
                            for mb in range(NB):
                                pA = ps2.tile([P, HD], f32, tag="pp")
                                n = 0
                                for b in range(B):
                                    for sb in range(NB):
                                        nc.tensor.matmul(
                                            pA,
                                            lhsT=skT[bs_of(b, j)][:, sb, mb * P:(mb + 1) * P],
                                            rhs=skT[bs_of(b, j)][:, sb, :],
                                            start=(n == 0), stop=(n == B * NB - 1),
                                        )
                                        n += 1
                                # negate: A-part = -K
                                nc.scalar.activation(at_t[:, mb, :], pA, AF.Copy, scale=-1.0)
                            if j > 0:
                                state["at1"] = at_t

            if SUB >= 5:
                at1 = state["at1"]
                bt1 = state["bt1"]
                # --- pair composition: abA = Abar^T = A0 A1 + A0 + A1 ---
                abA = pab.tile([P, NB, HD], bf, tag="abA")
                abB = pab.tile([P, NB, HD], bf, tag="abB")
                for mb in range(NB):
                    pA = ps2.tile([P, HD], f32, tag="pp")
                    for kb in range(NB):
                        nc.tensor.matmul(
                            pA, lhsT=AT0[:, kb, mb * P:(mb + 1) * P], rhs=at1[:, kb, :],
                            start=(kb == 0), stop=False,
                        )
                    nc.tensor.matmul(pA, lhsT=ID, rhs=AT0[:, mb, :], start=False, stop=False)
                    nc.tensor.matmul(pA, lhsT=ID, rhs=at1[:, mb, :], start=False, stop=True)
                    nc.scalar.activation(abA[:, mb, :], pA, AF.Copy)
                for mb in range(NB):
                    pB = ps2.tile([P, HD], f32, tag="pp")
                    for kb in range(NB):
                        nc.tensor.matmul(
                            pB, lhsT=at1[:, kb, mb * P:(mb + 1) * P], rhs=BT0[:, kb, :],
                            start=(kb == 0), stop=False,
                        )
                    nc.tensor.matmul(pB, lhsT=ID, rhs=BT0[:, mb, :], start=False, stop=False)
                    nc.tensor.matmul(pB, lhsT=ID, rhs=bt1[:, mb, :], start=False, stop=True)
                    nc.scalar.activation(abB[:, mb, :], pB, AF.Copy)
                nc.sync.dma_start(out=ab_in[0].rearrange("(kb p) n -> p kb n", p=P), in_=abA)
                nc.sync.dma_start(out=ab_in[1].rearrange("(kb p) n -> p kb n", p=P), in_=abB)

            # ---- AG2: pair compositions (flies behind attention) ----
            if STAGE >= 5:
                nc.gpsimd.collective_compute(
                    "AllGather", OP.bypass,
                    replica_groups=[list(range(NC))],
                    ins=[ab_in.opt()], outs=[ab_out.opt()],
                )

            # ---- q/kT + attention for the remaining units ----
            for j in range(SPC):
                for b in range(B):
                    if (b, j) != (0, 0):
                        unit_qk_attention(b, j)

        if STAGE >= 6:
            # ============ phase B prep (independent of AG2), then chain,
            # then memread + combine + Wd ============
            nc.vector.memset(MSEL, 0.0)
            with tc.tile_pool(name="pst", bufs=BS) as pst, \
                 tc.tile_pool(name="pb", bufs=2) as pb, \
                 tc.tile_pool(name="pch", bufs=2) as pch, \
                 tc.tile_pool(name="psch", bufs=NB, space="PSUM") as psch, \
                 tc.tile_pool(name="psb", bufs=2, space="PSUM") as psb, \
                 tc.tile_pool(name="psw", bufs=2, space="PSUM") as psw:

                # ---- prep: load attention terms, memread denominators ----
                # (denominators staged to DRAM, one batched [128,32] exact
                # reciprocal, then per-unit partition broadcasts; st loads
                # ride the scalar queue so they start as soon as each unit's
                # attention lands in DRAM)
                stT = [None] * BS
                rcmB = [None] * BS
                mstg = pb.tile([1, BS, SEG], bf, tag="mstg", bufs=1)
                for j in range(SPC):
                    for b in range(B):
                        i = bs_of(b, j)
                        st_i = pst.tile([P, NB, SEG], bf, tag="stp2", name=f"stp2_{i}")
                        stT[i] = st_i
                        nc.sync.dma_start(
                            out=st_i, in_=step_d[i].rearrange("kb p s -> p kb s"))
                        sq_i = sqT[i]
                        pd = psw.tile([1, SEG], f32, tag="dn", name="pd")
                        for kb in range(NB):
                            nc.tensor.matmul(
                                pd, lhsT=ZCOL[:, kb, i:i + 1], rhs=sq_i[:, kb, :],
                                start=(kb == 0), stop=(kb == NB - 1),
                            )
                        nc.scalar.activation(mstg[0:1, i, :], pd, AF.Copy)
                nc.sync.dma_start(out=rcm_d, in_=mstg[0:1, :, :])
                mrs = pb.tile([P, 32], bf, tag="mrs", bufs=1)
                nc.sync.dma_start(
                    out=mrs, in_=rcm_d.rearrange("i (a f) -> (i a) f", f=32))
                mrr = pb.tile([P, 32], f32, tag="mrr", bufs=1)
                nc.vector.reciprocal(mrr, mrs)
                nc.sync.dma_start(
                    out=rcmr_d.rearrange("i (a f) -> (i a) f", f=32), in_=mrr)
                for i in range(BS):
                    rcmb = pst.tile([P, SEG], f32, tag="rcmb", name=f"rcmb{i}")
                    rcmB[i] = rcmb
                    nc.gpsimd.dma_start(
                        out=rcmb,
                        in_=rcmr_d[i:i + 1, :].partition_broadcast(P))

                # ---- chain + select (cA/cB loads ride the gpsimd queue,
                # which is parked behind AG2 anyway) ----
                pM = [psch.tile([P, HD], f32, tag="ch", name=f"chain{i}") for i in range(NB)]
                mprev = None
                for step in range(NC - 1):
                    cA = pch.tile([P, NB, HD], bf, tag="cA")
                    cB = pch.tile([P, NB, HD], bf, tag="cB")
                    nc.gpsimd.dma_start(
                        out=cA, in_=ab_out[step, 0].rearrange("(kb p) n -> p kb n", p=P))
                    nc.gpsimd.dma_start(
                        out=cB, in_=ab_out[step, 1].rearrange("(kb p) n -> p kb n", p=P))
                    mcur = pch.tile([P, NB, HD], bf, tag="mc")
                    for mb in range(NB):
                        if step == 0:
                            nc.tensor.matmul(pM[mb], lhsT=ID, rhs=cB[:, mb, :],
                                             start=True, stop=True)
                        else:
                            for kb in range(NB):
                                nc.tensor.matmul(
                                    pM[mb], lhsT=cA[:, kb, mb * P:(mb + 1) * P],
                                    rhs=mprev[:, kb, :],
                                    start=False, stop=False,
                                )
                            nc.tensor.matmul(pM[mb], lhsT=ID, rhs=cB[:, mb, :],
                                             start=False, stop=True)
                        nc.scalar.activation(mcur[:, mb, :], pM[mb], AF.Copy)
                        nc.vector.scalar_tensor_tensor(
                            out=MSEL[:, mb, :], in0=mcur[:, mb, :],
                            scalar=OH[:, step:step + 1], in1=MSEL[:, mb, :],
                            op0=OP.mult, op1=OP.add,
                        )
                    mprev = mcur

                # M at segment 2c+1 = M + A0-part @ M + B0
                for mb in range(NB):
                    pm = psb.tile([P, HD], f32, tag="mm")
                    for kb in range(NB):
                        nc.tensor.matmul(
                            pm, lhsT=AT0[:, kb, mb * P:(mb + 1) * P], rhs=MSEL[:, kb, :],
                            start=(kb == 0), stop=False,
                        )
                    nc.tensor.matmul(pm, lhsT=ID, rhs=MSEL[:, mb, :], start=False, stop=False)
                    nc.tensor.matmul(pm, lhsT=ID, rhs=BT0[:, mb, :], start=False, stop=True)
                    nc.scalar.activation(MLOC1[:, mb, :], pm, AF.Copy)

                # ---- memread, combine, Wd ----
                for j in range(SPC):
                    Mt = MSEL if j == 0 else MLOC1
                    for b in range(B):
                        i = bs_of(b, j)
                        st_i = stT[i]
                        sq_i = sqT[i]
                        rcmb = rcmB[i]
                        for mb in range(NB):
                            pm = psb.tile([P, SEG], f32, tag="mm")
                            for kb in range(NB):
                                nc.tensor.matmul(
                                    pm, lhsT=Mt[:, kb, mb * P:(mb + 1) * P],
                                    rhs=sq_i[:, kb, :],
                                    start=(kb == 0), stop=(kb == NB - 1),
                                )
                            mtmp = pb.tile([P, SEG], bf, tag="mt")
                            nc.vector.scalar_tensor_tensor(
                                out=mtmp, in0=pm, scalar=GC[:, mb:mb + 1],
                                in1=rcmb,
                                op0=OP.mult, op1=OP.mult,
                            )
                            nc.vector.tensor_add(st_i[:, mb, :], st_i[:, mb, :], mtmp)
                        for sb in range(NB):
                            po = psw.tile([P, D], f32, tag="dn", name="po")
                            for mb in range(NB):
                                nc.tensor.matmul(
                                    po, lhsT=st_i[:, mb, sb * P:(sb + 1) * P],
                                    rhs=WD[:, mb, :],
                                    start=(mb == 0), stop=(mb == NB - 1),
                                )
                            ob = pb.tile([P, D], f32, tag="ob")
                            nc.vector.tensor_copy(ob, po)
                            nc.sync.dma_start(
                                out=out_d[b, j, sb * P:(sb + 1) * P, :], in_=ob)

    nc.compile()
    return nc


def _prep_inputs(x, Wq, Wk, Wv, Wd, beta):
    """Host-side prep: transpose/cast/shard. Returns in_maps (list of 8 dicts)."""
    g = 1.0 / (1.0 + np.exp(-beta.astype(np.float64)))  # (H,)
    g = g.astype(np.float32)
    gcol = np.repeat(g, D).reshape(NB, P).T.copy()      # (P, NB): g[(kb*128+p)//64]
    omg = (1.0 - np.repeat(g, D)).reshape(NB, P).T.copy()

    def wprep(w):
        return np.ascontiguousarray(
            w.reshape(NB, P, w.shape[1]).astype(bf_np))

    wq_a, wk_a, wv_a = wprep(Wq), wprep(Wk), wprep(Wv)
    wd_a = wprep(Wd)
    cmask = np.triu(np.ones((P, P), np.float32)).astype(bf_np)
    ident = np.eye(P, dtype=np.float32).astype(bf_np)

    # x -> per-core transposed blocks: xt[b, j, kb, p, s] = x[b, (2c+j)*SEG+s, kb*P+p]
    xs = x.reshape(B, NSEG, SEG, DIN)
    in_maps = []
    for c in range(NC):
        xloc = xs[:, 2 * c:2 * c + 2]                        # (B, SPC, SEG, DIN)
        xt = xloc.transpose(0, 1, 3, 2)                      # (B, SPC, DIN, SEG)
        xt = np.ascontiguousarray(
            xt.reshape(B, SPC, NB, P, SEG).astype(bf_np))
        # AG1 global row for (t, b): rank t//2 contributes row (t%2)*B + b
        zmask = np.zeros((64, NC), np.float32)
        for jj in range(NC):
            tgt = 2 * c + (jj // B)
            bb = jj % B
            for t in range(NSEG):
                if t < tgt:
                    zmask[(t // 2) * BS + (t % 2) * B + bb, jj] = 1.0
        oh = np.zeros((P, NC), np.float32)
        if c >= 1:
            oh[:, c - 1] = 1.0
        in_maps.append({
            "xt": xt, "wq": wq_a, "wk": wk_a, "wv": wv_a, "wd": wd_a,
            "gcol": gcol, "omg": omg, "zmask": zmask, "oh": oh,
            "cmask": cmask, "ident": ident,
        })
    return in_maps


def kernel(x, Wq, Wk, Wv, Wd, beta, _trace=False):
    x = np.asarray(x, np.float32)
    in_maps = _prep_inputs(
        x, np.asarray(Wq, np.float32), np.asarray(Wk, np.float32),
        np.asarray(Wv, np.float32), np.asarray(Wd, np.float32),
        np.asarray(beta, np.float32))
    if "nc" not in _CACHE:
        _CACHE["nc"] = _build()
    nc = _CACHE["nc"]
    res = bass_utils.run_bass_kernel_spmd(
        nc, in_maps, core_ids=list(range(NC)), trace=_trace)
    _CACHE["last_results"] = res
    out = np.empty((B, L, D), np.float32)
    for c in range(NC):
        oc = res.results[c]["out"]                  # (B, SPC, SEG, D)
        out[:, 2 * c * SEG:(2 * c + 2) * SEG, :] = oc.reshape(B, SPC * SEG, D)
    return out
